# revision 21
# baseline (speedup 1.0000x reference)
"""DiT forward pass on 8 Trainium2 NeuronCores (Bass/Tile).

Sharding: token-parallel. Cores 0-3 take batch 0, cores 4-7 batch 1; each
core owns 256 contiguous tokens of its batch element. All weights are
replicated (bf16). The only cross-core communication is one AllGather per
layer of the RoPE'd self-attention K and V chunks within each 4-core group.

Layout: activations are kept feature-major ("transposed", [feature(part),
token(free)]) so every linear layer is a natural lhsT=W^T / rhs=x_T matmul
with zero on-device transposes. V (and cross-attn V) are produced in
token-major layout directly by swapping matmul operand roles, which is what
the attention P@V contraction needs. Softmax statistics (which reduce over
the partition dim in this layout) are computed with ones-vector matmuls on
the TensorEngine; no max-subtraction is needed because LN-bounded QK^T/8
logits are small.
"""

import numpy as np
import ml_dtypes

import concourse.bass as bass
import concourse.tile as tile
from concourse import bacc, mybir
from concourse.bass_utils import run_bass_kernel_spmd

F32 = mybir.dt.float32
F32R = mybir.dt.float32r
BF16 = mybir.dt.bfloat16
AF = mybir.ActivationFunctionType

N_CORES = 8
GROUPS = [[0, 1, 2, 3], [4, 5, 6, 7]]
D, H, HD, L = 768, 12, 64, 12
B, S, T_TXT = 2, 1024, 128
DF = 4 * D            # 3072
DFH = DF // 2         # mlp processed in two halves of 1536
TOK = S // 4          # 256 local tokens per core
FT = D // 128         # 6 feature tiles of d_model
FTH = DFH // 128      # 12 feature tiles per mlp half
NKT = S // 128        # 8 key tiles (full sequence)
PAIRS = H // 2        # 6 head pairs (2 heads per 128-partition tile)
KV_CHUNK = D * TOK    # elements of one k_T chunk == one v chunk

_CACHE = {}


def r(ap):
    """view an f32 AP as f32r for full-rate fp32 matmul"""
    return ap.bitcast(F32R)


def build_program(n_layers, weights):
    nc = bacc.Bacc("TRN2", target_bir_lowering=False, debug=False,
                   num_devices=N_CORES)

    def din(name, shape, dt=F32):
        return nc.dram_tensor(name, shape, dt, kind="ExternalInput").ap()

    g = {}
    g["x_t"] = din("x_t", [D, TOK])
    g["text_t"] = din("text_t", [D, T_TXT], BF16)
    g["ada_scale"] = din("ada_scale", [D])
    g["ada_shift"] = din("ada_shift", [D])
    g["cos_t"] = din("cos_t", [128, TOK])
    g["sin_t"] = din("sin_t", [128, TOK])
    # weights are baked into the NEFF as Const tensors: loaded to device
    # HBM once at model load, no per-execute input staging
    for nm, arr in weights.items():
        g[nm] = nc.inline_tensor(arr, name=nm).ap()
    g["out_t"] = nc.dram_tensor("out_t", [D, TOK], F32,
                                kind="ExternalOutput").ap()

    from contextlib import ExitStack
    import os
    _ts = bool(os.environ.get("KERNEL_TRACE_SIM"))
    with tile.TileContext(nc, trace_sim=_ts) as tc, ExitStack() as ctx:
        _build_body(nc, tc, n_layers, g, ctx)
    nc.compile()
    return nc


def _build_body(nc, tc, n_layers, g, ctx):
    out_t = g["out_t"]

    pool = lambda *a, **k: ctx.enter_context(tc.tile_pool(*a, **k))
    persist = pool(name="persist", bufs=1)
    acts = pool(name="acts", bufs=1)
    bfacts = pool(name="bfacts", bufs=2)
    wpool = pool(name="wpool", bufs=6)
    bias_pool = pool(name="bias", bufs=2)
    kvpool = pool(name="kv", bufs=1)
    tmp_pool = pool(name="tmp", bufs=2)
    row_pool = pool(name="rows", bufs=4)
    psA = pool(name="psA", bufs=3, space="PSUM")
    psS = pool(name="psS", bufs=2, space="PSUM")
    psO = pool(name="psO", bufs=1, space="PSUM")
    psR = pool(name="psR", bufs=2, space="PSUM")
    dram = pool(name="dram", bufs=1, space="DRAM")

    ones_bf = persist.tile([128, 128], BF16, name="ones_bf")
    nc.vector.memset(ones_bf[:], 1.0)
    eps_t = persist.tile([1, 1], F32, name="eps_t")
    nc.vector.memset(eps_t[:], 1e-5)

    cos_sb = persist.tile([128, TOK], F32, name="cos_sb")
    nc.sync.dma_start(cos_sb[:], g["cos_t"][:])
    sin_sb = persist.tile([128, TOK], F32, name="sin_sb")
    nc.sync.dma_start(sin_sb[:], g["sin_t"][:])

    text_bf = persist.tile([128, FT * T_TXT], BF16, name="text_bf")
    for kt in range(FT):
        nc.sync.dma_start(text_bf[:, kt * T_TXT:(kt + 1) * T_TXT],
                          g["text_t"][kt * 128:(kt + 1) * 128, :])

    def col_load(dram_vec, n, name):
        t = bias_pool.tile([128, n], F32, name=name, tag="col", bufs=16)
        nc.sync.dma_start(t[:], dram_vec.rearrange("(t p) -> p t", p=128))
        return t

    def pack_load(dram_sl, n, name, tag="pack", bufs=3):
        t = bias_pool.tile([128, n], F32, name=name, tag=tag, bufs=bufs)
        nc.sync.dma_start(t[:], dram_sl)
        return t

    def ln_t(h, gcol, gidx, bcol, bidx, out_dt, name, out_tag, out_pool,
             out_bufs=None):
        """LayerNorm over features (partitions) of wide h [128, 6*TOK] f32."""
        mean_ps = psR.tile([1, TOK], F32, name=f"{name}_mps", tag="rowps")
        m2_ps = psR.tile([1, TOK], F32, name=f"{name}_m2ps", tag="rowps")
        for kt in range(FT):
            hb = tmp_pool.tile([128, TOK], BF16, name=f"{name}_hb",
                               tag="sqtmp", bufs=4)
            nc.vector.tensor_copy(hb[:], h[:, kt * TOK:(kt + 1) * TOK])
            nc.tensor.matmul(mean_ps[:], ones_bf[:, 0:1], hb[:],
                             start=(kt == 0), stop=(kt == FT - 1))
            h2 = tmp_pool.tile([128, TOK], BF16, name=f"{name}_h2",
                               tag="sqtmp", bufs=4)
            nc.scalar.square(h2[:], h[:, kt * TOK:(kt + 1) * TOK])
            nc.tensor.matmul(m2_ps[:], ones_bf[:, 0:1], h2[:],
                             start=(kt == 0), stop=(kt == FT - 1))
        mean = row_pool.tile([1, TOK], F32, name=f"{name}_mean", tag="row")
        nc.scalar.mul(mean[:], mean_ps[:], 1.0 / D)
        var = row_pool.tile([1, TOK], F32, name=f"{name}_var", tag="row")
        nc.vector.tensor_mul(var[:], mean[:], mean[:])
        m2 = row_pool.tile([1, TOK], F32, name=f"{name}_m2", tag="row")
        nc.scalar.mul(m2[:], m2_ps[:], 1.0 / D)
        nc.vector.tensor_sub(var[:], m2[:], var[:])
        std = row_pool.tile([1, TOK], F32, name=f"{name}_std", tag="row")
        nc.scalar.activation(std[:], var[:], AF.Sqrt, bias=eps_t[:])
        rstd = row_pool.tile([1, TOK], F32, name=f"{name}_rstd", tag="row")
        nc.vector.reciprocal(rstd[:], std[:])
        mean_bf = row_pool.tile([1, TOK], BF16, name=f"{name}_meanbf",
                                tag="rowbf", bufs=4)
        nc.vector.tensor_copy(mean_bf[:], mean[:])
        rstd_bf = row_pool.tile([1, TOK], BF16, name=f"{name}_rstdbf",
                                tag="rowbf", bufs=4)
        nc.vector.tensor_copy(rstd_bf[:], rstd[:])
        mean_bc = psS.tile([128, TOK], F32, name=f"{name}_mbc", tag="sps")
        nc.tensor.matmul(mean_bc[:], ones_bf[0:1, :], mean_bf[:])
        rstd_bc = psS.tile([128, TOK], F32, name=f"{name}_rbc", tag="sps")
        nc.tensor.matmul(rstd_bc[:], ones_bf[0:1, :], rstd_bf[:])
        out = out_pool.tile([128, FT * TOK], out_dt, name=name, tag=out_tag,
                            bufs=out_bufs)
        for kt in range(FT):
            t1 = tmp_pool.tile([128, TOK], F32, name=f"{name}_t1", tag="lntmp")
            nc.vector.tensor_sub(t1[:], h[:, kt * TOK:(kt + 1) * TOK],
                                 mean_bc[:])
            nc.vector.tensor_mul(t1[:], t1[:], rstd_bc[:])
            nc.scalar.activation(out[:, kt * TOK:(kt + 1) * TOK], t1[:],
                                 AF.Identity,
                                 bias=bcol[:, bidx + kt:bidx + kt + 1],
                                 scale=gcol[:, gidx + kt:gidx + kt + 1])
        return out

    def rope_tile(sk, dst, name):
        rot = tmp_pool.tile([128, TOK], F32, name=f"{name}_rot", tag="rope")
        for h0 in (0, 64):
            nc.vector.tensor_scalar_mul(
                rot[h0:h0 + 32, :], sk[h0 + 32:h0 + 64, :], -1.0)
            nc.vector.tensor_copy(rot[h0 + 32:h0 + 64, :], sk[h0:h0 + 32, :])
        t1 = tmp_pool.tile([128, TOK], F32, name=f"{name}_t1", tag="rope")
        nc.vector.tensor_mul(t1[:], sk, cos_sb[:])
        nc.vector.tensor_mul(rot[:], rot[:], sin_sb[:])
        nc.vector.tensor_add(dst, t1[:], rot[:])

    def attention(q_bf, get_k, v_bf, nkt, klen, name):
        """q_bf wide [128, 6*TOK] bf16; get_k(p) -> [128, klen] bf16 tile for
        head pair p; v_bf wide [128, nkt*D] bf16 (token-major).
        Returns softmax(qk/8)@v, feature-major wide [128, 6*TOK] bf16."""
        o_norm = bfacts.tile([128, FT * TOK], BF16, name=name, tag="onorm")
        kblk = klen // nkt
        for p in range(PAIRS):
            kp = get_k(p)
            o_ps = psO.tile([128, TOK], F32, name=f"{name}_o", tag="ops")
            den_e = psR.tile([1, TOK], F32, name=f"{name}_de", tag="rowps")
            den_o = psR.tile([1, TOK], F32, name=f"{name}_do", tag="rowps")
            for kt0 in range(0, nkt, 2):
                kts = [kt0] if nkt == 1 else [kt0, kt0 + 1]
                for which, h0 in ((0, 0), (1, 64)):
                    s_ps = psS.tile([128, 2 * TOK], F32, name=f"{name}_s",
                                    tag="sps")
                    for i, kt in enumerate(kts):
                        nc.tensor.matmul(
                            s_ps[0:kblk, i * TOK:(i + 1) * TOK],
                            kp[h0:h0 + 64, kt * kblk:(kt + 1) * kblk],
                            q_bf[h0:h0 + 64, p * TOK:(p + 1) * TOK])
                    e = tmp_pool.tile([128, 2 * TOK], BF16, name=f"{name}_e",
                                      tag="exps", bufs=4)
                    nw = len(kts) * TOK
                    nc.scalar.activation(e[0:kblk, 0:nw], s_ps[0:kblk, 0:nw],
                                         AF.Exp, scale=0.125)
                    hh = (2 * p + which) * HD
                    for i, kt in enumerate(kts):
                        nc.tensor.matmul(
                            o_ps[h0:h0 + 64, :],
                            v_bf[0:kblk, kt * D + hh: kt * D + hh + HD],
                            e[0:kblk, i * TOK:(i + 1) * TOK],
                            start=(kt == 0), stop=(kt == nkt - 1))
                        nc.tensor.matmul(
                            (den_e if which == 0 else den_o)[:],
                            ones_bf[0:kblk, 0:1], e[0:kblk, i * TOK:(i + 1) * TOK],
                            start=(kt == 0), stop=(kt == nkt - 1))
            rec_e = row_pool.tile([1, TOK], F32, name=f"{name}_re", tag="row")
            nc.vector.reciprocal(rec_e[:], den_e[:])
            rec_o = row_pool.tile([1, TOK], F32, name=f"{name}_ro", tag="row")
            nc.vector.reciprocal(rec_o[:], den_o[:])
            reb = row_pool.tile([1, TOK], BF16, name=f"{name}_reb",
                                tag="rowbf", bufs=4)
            nc.scalar.copy(reb[:], rec_e[:])
            rob = row_pool.tile([1, TOK], BF16, name=f"{name}_rob",
                                tag="rowbf", bufs=4)
            nc.scalar.copy(rob[:], rec_o[:])
            rb_ps = psS.tile([128, TOK], F32, name=f"{name}_rb", tag="sps")
            nc.tensor.matmul(rb_ps[0:64, :], ones_bf[0:1, 0:64], reb[:])
            nc.tensor.matmul(rb_ps[64:128, :], ones_bf[0:1, 0:64], rob[:])
            rb = tmp_pool.tile([128, TOK], F32, name=f"{name}_rbsb",
                               tag="rbsb")
            nc.vector.tensor_copy(rb[:], rb_ps[:])
            nc.vector.tensor_mul(o_norm[:, p * TOK:(p + 1) * TOK], o_ps[:],
                                 rb[:])
        return o_norm

    # ---------------- prologue ----------------
    sc_col = col_load(g["ada_scale"], FT, "sc_col")
    sh_col = col_load(g["ada_shift"], FT, "sh_col")
    ppk = pack_load(g["ppack"], 12, "ppack", tag="pack")
    pib_col = ppk[:, 0:FT]

    x_sb = acts.tile([128, FT * TOK], F32, name="x_sb", tag="h", bufs=3)
    for kt in range(FT):
        nc.sync.dma_start(x_sb[:, kt * TOK:(kt + 1) * TOK],
                          g["x_t"][kt * 128:(kt + 1) * 128, :])
    x_bf = bfacts.tile([128, FT * TOK], BF16, name="x_bf", tag="xn")
    for kt in range(FT):
        nc.scalar.copy(x_bf[:, kt * TOK:(kt + 1) * TOK],
                       x_sb[:, kt * TOK:(kt + 1) * TOK])

    # x_skip = proj_in(x); spilled to DRAM until the epilogue
    xskip_dram = dram.tile([D, TOK], F32, name="xskip_dram")
    piww = wpool.tile([128, FT * D], BF16, name="piw_sb", tag="wsmall",
                      bufs=1)
    nc.sync.dma_start(piww[:].rearrange("p (kt c) -> p kt c", kt=FT),
                      g["piw"].rearrange("(kt p) c -> p kt c", p=128))
    for ft in range(FT):
        ps = psA.tile([128, TOK], F32, name="pi_ps", tag="mmps")
        for kt in range(FT):
            nc.tensor.matmul(ps[:],
                             piww[:, kt * D + ft * 128: kt * D + (ft + 1) * 128],
                             x_bf[:, kt * TOK:(kt + 1) * TOK],
                             start=(kt == 0), stop=(kt == FT - 1))
        t = tmp_pool.tile([128, TOK], F32, name="pi_d", tag="drain", bufs=4)
        nc.scalar.activation(t[:], ps[:], AF.Identity,
                             bias=pib_col[:, ft:ft + 1])
        nc.sync.dma_start(xskip_dram[ft * 128:(ft + 1) * 128, :], t[:])

    h = ln_t(x_sb, sc_col, 0, sh_col, 0, F32, "h0", "h", acts, out_bufs=3)

    # ---------------- layers ----------------
    # packed per-layer bias columns, see _prep_weights for the layout
    OQKVB, OCAB, OCAOB, OF1B, OGB, OF2B = 0, 18, 36, 42, 66, 90
    ON1G, ON1B, ON2G, ON2B, ON3G, ON3B = 96, 102, 108, 114, 120, 126
    NBC = 132
    for li in range(n_layers):
        bcol = pack_load(g["bpack"][li], NBC, f"bpack{li}")
        qkvb_col = bcol[:, OQKVB:OQKVB + 18]
        n1g_col = bcol[:, ON1G:ON1G + FT]
        n1b_col = bcol[:, ON1B:ON1B + FT]

        # --- self-attention ---
        xn = ln_t(h, n1g_col, 0, n1b_col, 0, BF16, f"xn1_{li}", "xn", bfacts)

        qkvw_l = g["qkvw"][li]
        wts = []
        for kt in range(FT):
            wt = wpool.tile([128, 3 * D], BF16, name=f"qkvw{li}_{kt}",
                            tag="wqkv", bufs=6)
            nc.sync.dma_start(wt[:], qkvw_l[kt * 128:(kt + 1) * 128, :])
            wts.append(wt)

        q_bf = bfacts.tile([128, FT * TOK], BF16, name=f"qr_{li}", tag="qbf")
        k_rope = bfacts.tile([128, FT * TOK], BF16, name=f"kr_{li}",
                             tag="krbf")
        for sec, dstw in ((0, q_bf), (1, k_rope)):
            for ft in range(FT):
                ps = psA.tile([128, TOK], F32, name=f"qk{li}", tag="mmps")
                col = sec * D + ft * 128
                for kt in range(FT):
                    nc.tensor.matmul(ps[:], wts[kt][:, col:col + 128],
                                     xn[:, kt * TOK:(kt + 1) * TOK],
                                     start=(kt == 0), stop=(kt == FT - 1))
                qf = tmp_pool.tile([128, TOK], F32, name=f"qkd{li}",
                                   tag="qkdrain", bufs=3)
                nc.scalar.activation(
                    qf[:], ps[:], AF.Identity,
                    bias=qkvb_col[:, 6 * sec + ft:6 * sec + ft + 1])
                rope_tile(qf[:], dstw[:, ft * TOK:(ft + 1) * TOK],
                          f"rope{li}_{sec}_{ft}")
        # v in token-major layout: v[tok, f] = sum_d xn_T[d, tok] Wv_T[d, f]
        vbias_sb = bias_pool.tile([128, D], F32, name=f"vb{li}", tag="vbias")
        nc.sync.dma_start(vbias_sb[:], g["vbias"][li])
        v_bf = bfacts.tile([128, 2 * D], BF16, name=f"v_{li}", tag="vloc")
        for half in range(2):
            for nch in range(2):
                ps = psA.tile([128, 384], F32, name=f"v{li}", tag="mmps")
                for kt in range(FT):
                    nc.tensor.matmul(
                        ps[:],
                        xn[:, kt * TOK + half * 128: kt * TOK + half * 128 + 128],
                        wts[kt][:, 2 * D + nch * 384: 2 * D + (nch + 1) * 384],
                        start=(kt == 0), stop=(kt == FT - 1))
                nc.vector.tensor_add(
                    v_bf[:, half * D + nch * 384: half * D + (nch + 1) * 384],
                    ps[:], vbias_sb[:, nch * 384:(nch + 1) * 384])

        # --- AllGather (k_rope, v) within the 4-core group ---
        snd = dram.tile([2 * KV_CHUNK], BF16, name=f"snd{li}", tag="snd")
        nc.sync.dma_start(
            snd[0:KV_CHUNK].rearrange("(kt p c) -> p kt c", kt=FT, p=128),
            k_rope[:].rearrange("p (kt c) -> p kt c", kt=FT))
        nc.sync.dma_start(
            snd[KV_CHUNK:2 * KV_CHUNK].rearrange("(hf p c) -> p hf c", hf=2,
                                                 p=128),
            v_bf[:].rearrange("p (hf c) -> p hf c", hf=2))
        rcv = dram.tile([8 * KV_CHUNK], BF16, name=f"rcv{li}", tag="rcv")
        nc.gpsimd.collective_compute(
            "AllGather", mybir.AluOpType.bypass, replica_groups=GROUPS,
            ins=[snd[:].opt()], outs=[rcv[:].opt()])

        v_all = kvpool.tile([128, NKT * D], BF16, name=f"vall{li}", tag="vall")
        # two 3D-AP DMAs (one per v half): src[rk, p, c] rank-major in rcv
        vsrc = rcv.rearrange("(rk hp p c) -> p rk hp c", rk=4, hp=4, p=128)
        vdst = v_all[:].rearrange("p (rk hf c) -> p rk hf c", rk=4, hf=2)
        for hf in range(2):
            nc.sync.dma_start(vdst[:, :, hf, :], vsrc[:, :, 2 + hf, :])

        def get_k(p, li=li, rcv=rcv):
            kp = kvpool.tile([128, S], BF16, name=f"kst{li}_{p}", tag="kst",
                             bufs=2)
            ksrc = rcv.rearrange("(rk half ft p c) -> p rk half ft c",
                                 rk=4, half=2, ft=FT, p=128)
            nc.sync.dma_start(
                kp[:].rearrange("p (rk c) -> p rk c", rk=4),
                ksrc[:, :, 0, p, :])
            return kp

        o_sa = attention(q_bf, get_k, v_all, NKT, S, f"sa{li}")
        h2 = acts.tile([128, FT * TOK], F32, name=f"h2_{li}", tag="h", bufs=3)
        for kt in range(FT):
            sl = slice(kt * TOK, (kt + 1) * TOK)
            nc.vector.tensor_add(h2[:, sl], h[:, sl], o_sa[:, sl])

        # --- cross-attention ---
        n2g_col = bcol[:, ON2G:ON2G + FT]
        n2b_col = bcol[:, ON2B:ON2B + FT]
        cab_col = bcol[:, OCAB:OCAB + 18]
        xn2 = ln_t(h2, n2g_col, 0, n2b_col, 0, BF16, f"xn2_{li}", "xn",
                   bfacts)

        caw_l = g["caw"][li]
        cwts = []
        for kt in range(FT):
            wt = wpool.tile([128, 3 * D], BF16, name=f"caw{li}_{kt}",
                            tag="wqkv", bufs=6)
            nc.sync.dma_start(wt[:], caw_l[kt * 128:(kt + 1) * 128, :])
            cwts.append(wt)

        caq_bf = bfacts.tile([128, FT * TOK], BF16, name=f"caq_{li}",
                             tag="qbf")
        for ft in range(FT):
            ps = psA.tile([128, TOK], F32, name=f"caq{li}", tag="mmps")
            for kt in range(FT):
                nc.tensor.matmul(ps[:], cwts[kt][:, ft * 128:(ft + 1) * 128],
                                 xn2[:, kt * TOK:(kt + 1) * TOK],
                                 start=(kt == 0), stop=(kt == FT - 1))
            nc.scalar.activation(caq_bf[:, ft * TOK:(ft + 1) * TOK], ps[:],
                                 AF.Identity, bias=cab_col[:, ft:ft + 1])
        cak_bf = bfacts.tile([128, FT * T_TXT], BF16, name=f"cak_{li}",
                             tag="krbf")
        for ft in range(FT):
            ps = psA.tile([128, T_TXT], F32, name=f"cak{li}", tag="mmps")
            col = D + ft * 128
            for kt in range(FT):
                nc.tensor.matmul(ps[0:128, :], cwts[kt][:, col:col + 128],
                                 text_bf[:, kt * T_TXT:(kt + 1) * T_TXT],
                                 start=(kt == 0), stop=(kt == FT - 1))
            nc.scalar.activation(cak_bf[:, ft * T_TXT:(ft + 1) * T_TXT],
                                 ps[0:128, :], AF.Identity,
                                 bias=cab_col[:, 6 + ft:7 + ft])
        cavb_sb = bias_pool.tile([128, D], F32, name=f"cavb{li}", tag="vbias")
        nc.sync.dma_start(cavb_sb[:], g["cavbias"][li])
        cav_bf = bfacts.tile([128, D], BF16, name=f"cav_{li}", tag="vloc")
        for nch in range(2):
            ps = psA.tile([128, 384], F32, name=f"cav{li}", tag="mmps")
            for kt in range(FT):
                nc.tensor.matmul(
                    ps[:], text_bf[:, kt * T_TXT:(kt + 1) * T_TXT],
                    cwts[kt][:, 2 * D + nch * 384: 2 * D + (nch + 1) * 384],
                    start=(kt == 0), stop=(kt == FT - 1))
            nc.vector.tensor_add(cav_bf[:, nch * 384:(nch + 1) * 384], ps[:],
                                 cavb_sb[:, nch * 384:(nch + 1) * 384])

        o_ca = attention(caq_bf,
                         lambda p: cak_bf[:, p * T_TXT:(p + 1) * T_TXT],
                         cav_bf, 1, T_TXT, f"ca{li}")

        caob_col = bcol[:, OCAOB:OCAOB + FT]
        oww = wpool.tile([128, FT * D], BF16, name=f"caow{li}",
                         tag="wsmall", bufs=1)
        nc.sync.dma_start(
            oww[:].rearrange("p (kt c) -> p kt c", kt=FT),
            g["caow"][li].rearrange("(kt p) c -> p kt c", p=128))
        h3 = acts.tile([128, FT * TOK], F32, name=f"h3_{li}", tag="h", bufs=3)
        for ft in range(FT):
            ps = psA.tile([128, TOK], F32, name=f"cao{li}", tag="mmps")
            for kt in range(FT):
                nc.tensor.matmul(ps[:],
                                 oww[:, kt * D + ft * 128:
                                     kt * D + (ft + 1) * 128],
                                 o_ca[:, kt * TOK:(kt + 1) * TOK],
                                 start=(kt == 0), stop=(kt == FT - 1))
            t = tmp_pool.tile([128, TOK], F32, name=f"cao_d{li}", tag="drain", bufs=4)
            nc.scalar.activation(t[:], ps[:], AF.Identity,
                                 bias=caob_col[:, ft:ft + 1])
            sl = slice(ft * TOK, (ft + 1) * TOK)
            nc.vector.tensor_add(h3[:, sl], h2[:, sl], t[:])

        # --- gated MLP (two DF halves, fc2 accumulated in SBUF) ---
        n3g_col = bcol[:, ON3G:ON3G + FT]
        n3b_col = bcol[:, ON3B:ON3B + FT]
        fc1b_col = bcol[:, OF1B:OF1B + 2 * FTH]
        gateb_col = bcol[:, OGB:OGB + 2 * FTH]
        fc2b_col = bcol[:, OF2B:OF2B + FT]
        xn3 = ln_t(h3, n3g_col, 0, n3b_col, 0, BF16, f"xn3_{li}", "xn",
                   bfacts)

        acc = acts.tile([128, FT * TOK], F32, name=f"acc_{li}", tag="acc",
                        bufs=1)
        for half in range(2):
            w1 = []
            wg = []
            for kt in range(FT):
                wt = wpool.tile([128, DFH], BF16, name=f"f1w{li}_{half}_{kt}",
                                tag="wmlp", bufs=7)
                nc.sync.dma_start(
                    wt[:], g["fc1w"][li][kt * 128:(kt + 1) * 128,
                                         half * DFH:(half + 1) * DFH])
                w1.append(wt)
            up_h = bfacts.tile([128, FTH * TOK], BF16, name=f"up{li}_{half}",
                               tag="up")
            for ft in range(FTH):
                ps = psA.tile([128, TOK], F32, name=f"f1{li}", tag="mmps")
                for kt in range(FT):
                    nc.tensor.matmul(ps[:],
                                     w1[kt][:, ft * 128:(ft + 1) * 128],
                                     xn3[:, kt * TOK:(kt + 1) * TOK],
                                     start=(kt == 0), stop=(kt == FT - 1))
                fi = half * FTH + ft
                nc.scalar.activation(up_h[:, ft * TOK:(ft + 1) * TOK], ps[:],
                                     AF.Gelu, bias=fc1b_col[:, fi:fi + 1])
            for kt in range(FT):
                wt = wpool.tile([128, DFH], BF16, name=f"gw{li}_{half}_{kt}",
                                tag="wmlp", bufs=7)
                nc.sync.dma_start(
                    wt[:], g["gatew"][li][kt * 128:(kt + 1) * 128,
                                          half * DFH:(half + 1) * DFH])
                wg.append(wt)
            gate_h = bfacts.tile([128, FTH * TOK], BF16,
                                 name=f"gate{li}_{half}", tag="gate", bufs=1)
            for ft in range(FTH):
                ps = psA.tile([128, TOK], F32, name=f"gt{li}", tag="mmps")
                for kt in range(FT):
                    nc.tensor.matmul(ps[:],
                                     wg[kt][:, ft * 128:(ft + 1) * 128],
                                     xn3[:, kt * TOK:(kt + 1) * TOK],
                                     start=(kt == 0), stop=(kt == FT - 1))
                fi = half * FTH + ft
                nc.scalar.activation(gate_h[:, ft * TOK:(ft + 1) * TOK],
                                     ps[:], AF.Sigmoid,
                                     bias=gateb_col[:, fi:fi + 1])
            for ft in range(FTH):
                sl = slice(ft * TOK, (ft + 1) * TOK)
                nc.vector.tensor_mul(up_h[:, sl], up_h[:, sl], gate_h[:, sl])
            w2w = wpool.tile([128, FTH * D], BF16, name=f"f2w{li}_{half}",
                             tag="wmlp2", bufs=1)
            nc.sync.dma_start(
                w2w[:].rearrange("p (kt c) -> p kt c", kt=FTH),
                g["fc2w"][li][half * DFH:(half + 1) * DFH, :].rearrange(
                    "(kt p) c -> p kt c", p=128))
            for ft in range(FT):
                ps = psA.tile([128, TOK], F32, name=f"f2{li}", tag="mmps")
                for kt in range(FTH):
                    nc.tensor.matmul(ps[:],
                                     w2w[:, kt * D + ft * 128:
                                         kt * D + (ft + 1) * 128],
                                     up_h[:, kt * TOK:(kt + 1) * TOK],
                                     start=(kt == 0), stop=(kt == FTH - 1))
                sl = slice(ft * TOK, (ft + 1) * TOK)
                if half == 0:
                    nc.scalar.activation(acc[:, sl], ps[:], AF.Identity,
                                         bias=fc2b_col[:, ft:ft + 1])
                else:
                    t = tmp_pool.tile([128, TOK], F32, name=f"f2d{li}",
                                      tag="drain", bufs=4)
                    nc.scalar.copy(t[:], ps[:])
                    nc.vector.tensor_add(acc[:, sl], acc[:, sl], t[:])

        h_new = acts.tile([128, FT * TOK], F32, name=f"h4_{li}", tag="h",
                          bufs=3)
        for ft in range(FT):
            sl = slice(ft * TOK, (ft + 1) * TOK)
            nc.vector.tensor_add(h_new[:, sl], h3[:, sl], acc[:, sl])
        h = h_new

    # ---------------- epilogue ----------------
    pob_col = ppk[:, FT:2 * FT]
    h_bf = bfacts.tile([128, FT * TOK], BF16, name="h_final", tag="xn")
    for kt in range(FT):
        sl = slice(kt * TOK, (kt + 1) * TOK)
        nc.scalar.copy(h_bf[:, sl], h[:, sl])
    poww = wpool.tile([128, FT * D], BF16, name="pow_sb", tag="wsmall",
                      bufs=1)
    nc.sync.dma_start(poww[:].rearrange("p (kt c) -> p kt c", kt=FT),
                      g["pow"].rearrange("(kt p) c -> p kt c", p=128))
    for ft in range(FT):
        ps = psA.tile([128, TOK], F32, name="po_ps", tag="mmps")
        for kt in range(FT):
            nc.tensor.matmul(ps[:],
                             poww[:, kt * D + ft * 128: kt * D + (ft + 1) * 128],
                             h_bf[:, kt * TOK:(kt + 1) * TOK],
                             start=(kt == 0), stop=(kt == FT - 1))
        t = tmp_pool.tile([128, TOK], F32, name="po_d", tag="drain", bufs=4)
        nc.scalar.activation(t[:], ps[:], AF.Identity,
                             bias=pob_col[:, ft:ft + 1])
        xs = tmp_pool.tile([128, TOK], F32, name="po_xs", tag="drain", bufs=4)
        nc.sync.dma_start(xs[:], xskip_dram[ft * 128:(ft + 1) * 128, :])
        o = tmp_pool.tile([128, TOK], F32, name="po_o", tag="drain", bufs=4)
        nc.vector.tensor_add(o[:], xs[:], t[:])
        nc.sync.dma_start(out_t[ft * 128:(ft + 1) * 128, :], o[:])


# ---------------------------------------------------------------------------
# host side
# ---------------------------------------------------------------------------

def _silu(x):
    return x / (1.0 + np.exp(-x))


def _host_prep(inputs, n_layers=L):
    f32 = lambda a: np.ascontiguousarray(np.asarray(a, np.float32))
    bf = lambda a: np.ascontiguousarray(np.asarray(a, np.float32)).astype(
        ml_dtypes.bfloat16)

    x = f32(inputs["x"])
    text_emb = f32(inputs["text_emb"])
    t = np.asarray(inputs["t"])

    temb = f32(inputs["t_emb_table"])[t]
    temb = _silu(temb @ f32(inputs["te_w1"]).T + f32(inputs["te_b1"]))
    temb = temb @ f32(inputs["te_w2"]).T + f32(inputs["te_b2"])
    tmean = text_emb.mean(axis=1)
    ada_t = _silu(temb) @ f32(inputs["ada_time_w"]).T + f32(inputs["ada_time_b"])
    ada_x = _silu(tmean) @ f32(inputs["ada_text_w"]).T + f32(inputs["ada_text_b"])
    tsc, tsh = ada_t[:, :D], ada_t[:, D:]
    xsc, xsh = ada_x[:, :D], ada_x[:, D:]
    scale = (1.0 + tsc + xsc).astype(np.float32)
    shift = (tsh + xsh).astype(np.float32)

    inv_freq = 1.0 / (10000.0 ** (np.arange(0, HD, 2, dtype=np.float32) / HD))
    pos = np.arange(S, dtype=np.float32)[:, None] * inv_freq[None, :]
    ang = np.concatenate([pos, pos], axis=1)  # [S, 64]
    cos_t = np.tile(np.cos(ang).T, (2, 1)).astype(np.float32)  # [128, S]
    sin_t = np.tile(np.sin(ang).T, (2, 1)).astype(np.float32)

    lw = _prep_weights(inputs, n_layers)

    in_maps = []
    for c in range(N_CORES):
        b, j = c // 4, c % 4
        rows = slice(j * TOK, (j + 1) * TOK)
        m = {}
        m["x_t"] = np.ascontiguousarray(x[b, rows].T)
        m["text_t"] = np.ascontiguousarray(text_emb[b].T).astype(
            ml_dtypes.bfloat16)
        m["ada_scale"] = scale[b]
        m["ada_shift"] = shift[b]
        m["cos_t"] = np.ascontiguousarray(cos_t[:, rows])
        m["sin_t"] = np.ascontiguousarray(sin_t[:, rows])
        in_maps.append(m)
    return in_maps, lw


def _prep_weights(inputs, n_layers=L):
    f32 = lambda a: np.ascontiguousarray(np.asarray(a, np.float32))
    bf = lambda a: np.ascontiguousarray(np.asarray(a, np.float32)).astype(
        ml_dtypes.bfloat16)
    lw = {}
    tr = lambda w: np.ascontiguousarray(np.swapaxes(f32(w), -1, -2))
    bcast_bias = lambda b: np.ascontiguousarray(np.broadcast_to(
        f32(b)[:, None, :], (n_layers, 128, D))).astype(np.float32)
    lw["qkvw"] = bf(tr(inputs["qkv_w"][:n_layers]))
    lw["vbias"] = bcast_bias(inputs["qkv_b"][:n_layers, 2 * D:])
    lw["caw"] = bf(tr(inputs["ca_qkv_w"][:n_layers]))
    lw["cavbias"] = bcast_bias(inputs["ca_qkv_b"][:n_layers, 2 * D:])
    lw["caow"] = bf(tr(inputs["ca_out_w"][:n_layers]))
    lw["fc1w"] = bf(tr(inputs["fc1_w"][:n_layers]))
    lw["gatew"] = bf(tr(inputs["gate_w"][:n_layers]))
    lw["fc2w"] = bf(tr(inputs["fc2_w"][:n_layers]))
    lw["piw"] = bf(f32(inputs["proj_in_w"]).T)
    lw["pow"] = bf(f32(inputs["proj_out_w"]).T)

    def cols(vec):
        return np.asarray(vec, np.float32).reshape(-1, 128).T  # [128, k]

    packs = []
    for li in range(n_layers):
        packs.append(np.concatenate([
            cols(inputs["qkv_b"][li]),          # 18
            cols(inputs["ca_qkv_b"][li]),       # 18
            cols(inputs["ca_out_b"][li]),       # 6
            cols(inputs["fc1_b"][li]),          # 24
            cols(inputs["gate_b"][li]),         # 24
            cols(inputs["fc2_b"][li]),          # 6
            cols(inputs["n1_g"][li]), cols(inputs["n1_b"][li]),
            cols(inputs["n2_g"][li]), cols(inputs["n2_b"][li]),
            cols(inputs["n3_g"][li]), cols(inputs["n3_b"][li]),
        ], axis=1))
    lw["bpack"] = np.ascontiguousarray(np.stack(packs)).astype(np.float32)
    lw["ppack"] = np.ascontiguousarray(np.concatenate([
        cols(inputs["proj_in_b"]), cols(inputs["proj_out_b"])],
        axis=1)).astype(np.float32)
    return lw


def kernel(**inputs):
    n_layers = _CACHE.get("n_layers", L)
    in_maps, lw = _host_prep(inputs, n_layers)
    if "nc" not in _CACHE:
        _CACHE["nc"] = build_program(n_layers, lw)
    nc = _CACHE["nc"]
    res = run_bass_kernel_spmd(nc, in_maps, core_ids=list(range(N_CORES)),
                               **_CACHE.get("run_kwargs", {}))
    _CACHE["last_result"] = res
    out = np.empty((B, S, D), np.float32)
    for c in range(N_CORES):
        b, j = c // 4, c % 4
        out[b, j * TOK:(j + 1) * TOK] = res.results[c]["out_t"].T
    return out


# revision 22
# speedup vs baseline: 1.9396x; 1.9396x over previous
"""DiT forward pass on 8 Trainium2 NeuronCores (Bass/Tile).

Sharding: token-parallel. Cores 0-3 take batch 0, cores 4-7 batch 1; each
core owns 256 contiguous tokens of its batch element. All weights are
replicated (bf16). The only cross-core communication is one AllGather per
layer of the RoPE'd self-attention K and V chunks within each 4-core group.

Layout: activations are kept feature-major ("transposed", [feature(part),
token(free)]) so every linear layer is a natural lhsT=W^T / rhs=x_T matmul
with zero on-device transposes. V (and cross-attn V) are produced in
token-major layout directly by swapping matmul operand roles, which is what
the attention P@V contraction needs. Softmax statistics (which reduce over
the partition dim in this layout) are computed with ones-vector matmuls on
the TensorEngine; no max-subtraction is needed because LN-bounded QK^T/8
logits are small.
"""

import numpy as np
import ml_dtypes

import concourse.bass as bass
import concourse.tile as tile
from concourse import bacc, mybir
from concourse.bass_utils import run_bass_kernel_spmd

F32 = mybir.dt.float32
F32R = mybir.dt.float32r
BF16 = mybir.dt.bfloat16
AF = mybir.ActivationFunctionType

N_CORES = 8
GROUPS = [[0, 1, 2, 3], [4, 5, 6, 7]]
D, H, HD, L = 768, 12, 64, 12
B, S, T_TXT = 2, 1024, 128
DF = 4 * D            # 3072
DFH = DF // 2         # mlp processed in two halves of 1536
TOK = S // 4          # 256 local tokens per core
FT = D // 128         # 6 feature tiles of d_model
FTH = DFH // 128      # 12 feature tiles per mlp half
NKT = S // 128        # 8 key tiles (full sequence)
PAIRS = H // 2        # 6 head pairs (2 heads per 128-partition tile)
KV_CHUNK = D * TOK    # elements of one k_T chunk == one v chunk

_CACHE = {}
_no_cc = False


def r(ap):
    """view an f32 AP as f32r for full-rate fp32 matmul"""
    return ap.bitcast(F32R)


def build_program(n_layers, weights):
    nc = bacc.Bacc("TRN2", target_bir_lowering=False, debug=False,
                   num_devices=N_CORES)

    def din(name, shape, dt=F32):
        return nc.dram_tensor(name, shape, dt, kind="ExternalInput").ap()

    g = {}
    g["x_t"] = din("x_t", [D, TOK])
    g["text_t"] = din("text_t", [D, T_TXT], BF16)
    g["ada_scale"] = din("ada_scale", [D])
    g["ada_shift"] = din("ada_shift", [D])
    g["cos_t"] = din("cos_t", [128, TOK])
    g["sin_t"] = din("sin_t", [128, TOK])
    # weights are baked into the NEFF as Const tensors: loaded to device
    # HBM once at model load, no per-execute input staging
    for nm, arr in weights.items():
        g[nm] = nc.inline_tensor(arr, name=nm).ap()
    g["out_t"] = nc.dram_tensor("out_t", [D, TOK], F32,
                                kind="ExternalOutput").ap()

    from contextlib import ExitStack
    import os
    _ts = bool(os.environ.get("KERNEL_TRACE_SIM"))
    global _no_cc
    _no_cc = bool(os.environ.get("KERNEL_NO_CC"))
    with tile.TileContext(nc, trace_sim=_ts) as tc, ExitStack() as ctx:
        _build_body(nc, tc, n_layers, g, ctx)
    nc.compile()
    return nc


def _build_body(nc, tc, n_layers, g, ctx):
    out_t = g["out_t"]

    pool = lambda *a, **k: ctx.enter_context(tc.tile_pool(*a, **k))
    persist = pool(name="persist", bufs=1)
    acts = pool(name="acts", bufs=1)
    bfacts = pool(name="bfacts", bufs=2)
    wpool = pool(name="wpool", bufs=6)
    bias_pool = pool(name="bias", bufs=2)
    kvpool = pool(name="kv", bufs=1)
    tmp_pool = pool(name="tmp", bufs=2)
    row_pool = pool(name="rows", bufs=4)
    psA = pool(name="psA", bufs=3, space="PSUM")
    psS = pool(name="psS", bufs=2, space="PSUM")
    psO = pool(name="psO", bufs=1, space="PSUM")
    psR = pool(name="psR", bufs=2, space="PSUM")
    dram = pool(name="dram", bufs=1, space="DRAM")

    ones_bf = persist.tile([128, 128], BF16, name="ones_bf")
    nc.vector.memset(ones_bf[:], 1.0)
    eps_t = persist.tile([1, 1], F32, name="eps_t")
    nc.vector.memset(eps_t[:], 1e-5)

    cos_sb = persist.tile([128, TOK], F32, name="cos_sb")
    nc.sync.dma_start(cos_sb[:], g["cos_t"][:])
    sin_sb = persist.tile([128, TOK], F32, name="sin_sb")
    nc.sync.dma_start(sin_sb[:], g["sin_t"][:])

    text_bf = persist.tile([128, FT * T_TXT], BF16, name="text_bf")
    for kt in range(FT):
        nc.sync.dma_start(text_bf[:, kt * T_TXT:(kt + 1) * T_TXT],
                          g["text_t"][kt * 128:(kt + 1) * 128, :])

    def col_load(dram_vec, n, name):
        t = bias_pool.tile([128, n], F32, name=name, tag="col", bufs=16)
        nc.sync.dma_start(t[:], dram_vec.rearrange("(t p) -> p t", p=128))
        return t

    def pack_load(dram_sl, n, name, tag="pack", bufs=3):
        t = bias_pool.tile([128, n], F32, name=name, tag=tag, bufs=bufs)
        nc.sync.dma_start(t[:], dram_sl)
        return t

    def ln_t(h, gcol, gidx, bcol, bidx, out_dt, name, out_tag, out_pool,
             out_bufs=None):
        """LayerNorm over features (partitions) of wide h [128, 6*TOK] f32."""
        mean_ps = psR.tile([1, TOK], F32, name=f"{name}_mps", tag="rowps")
        m2_ps = psR.tile([1, TOK], F32, name=f"{name}_m2ps", tag="rowps")
        for kt in range(FT):
            hb = tmp_pool.tile([128, TOK], BF16, name=f"{name}_hb",
                               tag="sqtmp", bufs=4)
            nc.vector.tensor_copy(hb[:], h[:, kt * TOK:(kt + 1) * TOK])
            nc.tensor.matmul(mean_ps[:], ones_bf[:, 0:1], hb[:],
                             start=(kt == 0), stop=(kt == FT - 1))
            h2 = tmp_pool.tile([128, TOK], BF16, name=f"{name}_h2",
                               tag="sqtmp", bufs=4)
            nc.scalar.square(h2[:], h[:, kt * TOK:(kt + 1) * TOK])
            nc.tensor.matmul(m2_ps[:], ones_bf[:, 0:1], h2[:],
                             start=(kt == 0), stop=(kt == FT - 1))
        mean = row_pool.tile([1, TOK], F32, name=f"{name}_mean", tag="row")
        nc.scalar.mul(mean[:], mean_ps[:], 1.0 / D)
        var = row_pool.tile([1, TOK], F32, name=f"{name}_var", tag="row")
        nc.vector.tensor_mul(var[:], mean[:], mean[:])
        m2 = row_pool.tile([1, TOK], F32, name=f"{name}_m2", tag="row")
        nc.scalar.mul(m2[:], m2_ps[:], 1.0 / D)
        nc.vector.tensor_sub(var[:], m2[:], var[:])
        std = row_pool.tile([1, TOK], F32, name=f"{name}_std", tag="row")
        nc.scalar.activation(std[:], var[:], AF.Sqrt, bias=eps_t[:])
        rstd = row_pool.tile([1, TOK], F32, name=f"{name}_rstd", tag="row")
        nc.vector.reciprocal(rstd[:], std[:])
        mean_bf = row_pool.tile([1, TOK], BF16, name=f"{name}_meanbf",
                                tag="rowbf", bufs=4)
        nc.vector.tensor_copy(mean_bf[:], mean[:])
        rstd_bf = row_pool.tile([1, TOK], BF16, name=f"{name}_rstdbf",
                                tag="rowbf", bufs=4)
        nc.vector.tensor_copy(rstd_bf[:], rstd[:])
        mean_bc = psS.tile([128, TOK], F32, name=f"{name}_mbc", tag="sps")
        nc.tensor.matmul(mean_bc[:], ones_bf[0:1, :], mean_bf[:])
        rstd_bc = psS.tile([128, TOK], F32, name=f"{name}_rbc", tag="sps")
        nc.tensor.matmul(rstd_bc[:], ones_bf[0:1, :], rstd_bf[:])
        out = out_pool.tile([128, FT * TOK], out_dt, name=name, tag=out_tag,
                            bufs=out_bufs)
        for kt in range(FT):
            t1 = tmp_pool.tile([128, TOK], F32, name=f"{name}_t1", tag="lntmp")
            nc.vector.tensor_sub(t1[:], h[:, kt * TOK:(kt + 1) * TOK],
                                 mean_bc[:])
            nc.vector.tensor_mul(t1[:], t1[:], rstd_bc[:])
            nc.scalar.activation(out[:, kt * TOK:(kt + 1) * TOK], t1[:],
                                 AF.Identity,
                                 bias=bcol[:, bidx + kt:bidx + kt + 1],
                                 scale=gcol[:, gidx + kt:gidx + kt + 1])
        return out

    def rope_tile(sk, dst, name):
        rot = tmp_pool.tile([128, TOK], F32, name=f"{name}_rot", tag="rope")
        for h0 in (0, 64):
            nc.vector.tensor_scalar_mul(
                rot[h0:h0 + 32, :], sk[h0 + 32:h0 + 64, :], -1.0)
            nc.vector.tensor_copy(rot[h0 + 32:h0 + 64, :], sk[h0:h0 + 32, :])
        t1 = tmp_pool.tile([128, TOK], F32, name=f"{name}_t1", tag="rope")
        nc.vector.tensor_mul(t1[:], sk, cos_sb[:])
        nc.vector.tensor_mul(rot[:], rot[:], sin_sb[:])
        nc.vector.tensor_add(dst, t1[:], rot[:])

    def attention(q_bf, get_k, v_bf, nkt, klen, name):
        """q_bf wide [128, 6*TOK] bf16; get_k(p) -> [128, klen] bf16 tile for
        head pair p; v_bf wide [128, nkt*D] bf16 (token-major).
        Returns softmax(qk/8)@v, feature-major wide [128, 6*TOK] bf16."""
        o_norm = bfacts.tile([128, FT * TOK], BF16, name=name, tag="onorm")
        kblk = klen // nkt
        for p in range(PAIRS):
            kp = get_k(p)
            o_ps = psO.tile([128, TOK], F32, name=f"{name}_o", tag="ops")
            den_e = psR.tile([1, TOK], F32, name=f"{name}_de", tag="rowps")
            den_o = psR.tile([1, TOK], F32, name=f"{name}_do", tag="rowps")
            for kt0 in range(0, nkt, 2):
                kts = [kt0] if nkt == 1 else [kt0, kt0 + 1]
                for which, h0 in ((0, 0), (1, 64)):
                    s_ps = psS.tile([128, 2 * TOK], F32, name=f"{name}_s",
                                    tag="sps")
                    for i, kt in enumerate(kts):
                        nc.tensor.matmul(
                            s_ps[0:kblk, i * TOK:(i + 1) * TOK],
                            kp[h0:h0 + 64, kt * kblk:(kt + 1) * kblk],
                            q_bf[h0:h0 + 64, p * TOK:(p + 1) * TOK])
                    e = tmp_pool.tile([128, 2 * TOK], BF16, name=f"{name}_e",
                                      tag="exps", bufs=4)
                    nw = len(kts) * TOK
                    nc.scalar.activation(e[0:kblk, 0:nw], s_ps[0:kblk, 0:nw],
                                         AF.Exp, scale=0.125)
                    hh = (2 * p + which) * HD
                    for i, kt in enumerate(kts):
                        nc.tensor.matmul(
                            o_ps[h0:h0 + 64, :],
                            v_bf[0:kblk, kt * D + hh: kt * D + hh + HD],
                            e[0:kblk, i * TOK:(i + 1) * TOK],
                            start=(kt == 0), stop=(kt == nkt - 1))
                        nc.tensor.matmul(
                            (den_e if which == 0 else den_o)[:],
                            ones_bf[0:kblk, 0:1], e[0:kblk, i * TOK:(i + 1) * TOK],
                            start=(kt == 0), stop=(kt == nkt - 1))
            rec_e = row_pool.tile([1, TOK], F32, name=f"{name}_re", tag="row")
            nc.vector.reciprocal(rec_e[:], den_e[:])
            rec_o = row_pool.tile([1, TOK], F32, name=f"{name}_ro", tag="row")
            nc.vector.reciprocal(rec_o[:], den_o[:])
            reb = row_pool.tile([1, TOK], BF16, name=f"{name}_reb",
                                tag="rowbf", bufs=4)
            nc.scalar.copy(reb[:], rec_e[:])
            rob = row_pool.tile([1, TOK], BF16, name=f"{name}_rob",
                                tag="rowbf", bufs=4)
            nc.scalar.copy(rob[:], rec_o[:])
            rb_ps = psS.tile([128, TOK], F32, name=f"{name}_rb", tag="sps")
            nc.tensor.matmul(rb_ps[0:64, :], ones_bf[0:1, 0:64], reb[:])
            nc.tensor.matmul(rb_ps[64:128, :], ones_bf[0:1, 0:64], rob[:])
            rb = tmp_pool.tile([128, TOK], F32, name=f"{name}_rbsb",
                               tag="rbsb")
            nc.vector.tensor_copy(rb[:], rb_ps[:])
            nc.vector.tensor_mul(o_norm[:, p * TOK:(p + 1) * TOK], o_ps[:],
                                 rb[:])
        return o_norm

    # ---------------- prologue ----------------
    sc_col = col_load(g["ada_scale"], FT, "sc_col")
    sh_col = col_load(g["ada_shift"], FT, "sh_col")
    ppk = pack_load(g["ppack"], 12, "ppack", tag="pack")
    pib_col = ppk[:, 0:FT]

    x_sb = acts.tile([128, FT * TOK], F32, name="x_sb", tag="h", bufs=3)
    for kt in range(FT):
        nc.sync.dma_start(x_sb[:, kt * TOK:(kt + 1) * TOK],
                          g["x_t"][kt * 128:(kt + 1) * 128, :])
    x_bf = bfacts.tile([128, FT * TOK], BF16, name="x_bf", tag="xn")
    for kt in range(FT):
        nc.scalar.copy(x_bf[:, kt * TOK:(kt + 1) * TOK],
                       x_sb[:, kt * TOK:(kt + 1) * TOK])

    # x_skip = proj_in(x); spilled to DRAM until the epilogue
    xskip_dram = dram.tile([D, TOK], F32, name="xskip_dram")
    piww = wpool.tile([128, FT * D], BF16, name="piw_sb", tag="wsmall",
                      bufs=1)
    nc.sync.dma_start(piww[:].rearrange("p (kt c) -> p kt c", kt=FT),
                      g["piw"].rearrange("(kt p) c -> p kt c", p=128))
    for ft in range(FT):
        ps = psA.tile([128, TOK], F32, name="pi_ps", tag="mmps")
        for kt in range(FT):
            nc.tensor.matmul(ps[:],
                             piww[:, kt * D + ft * 128: kt * D + (ft + 1) * 128],
                             x_bf[:, kt * TOK:(kt + 1) * TOK],
                             start=(kt == 0), stop=(kt == FT - 1))
        t = tmp_pool.tile([128, TOK], F32, name="pi_d", tag="drain", bufs=4)
        nc.scalar.activation(t[:], ps[:], AF.Identity,
                             bias=pib_col[:, ft:ft + 1])
        nc.sync.dma_start(xskip_dram[ft * 128:(ft + 1) * 128, :], t[:])

    h = ln_t(x_sb, sc_col, 0, sh_col, 0, F32, "h0", "h", acts, out_bufs=3)

    # ---------------- layers ----------------
    # packed per-layer bias columns, see _prep_weights for the layout
    OQKVB, OCAB, OCAOB, OF1B, OGB, OF2B = 0, 18, 36, 42, 66, 90
    ON1G, ON1B, ON2G, ON2B, ON3G, ON3B = 96, 102, 108, 114, 120, 126
    NBC = 132
    for li in range(n_layers):
        bcol = pack_load(g["bpack"][li], NBC, f"bpack{li}")
        qkvb_col = bcol[:, OQKVB:OQKVB + 18]
        n1g_col = bcol[:, ON1G:ON1G + FT]
        n1b_col = bcol[:, ON1B:ON1B + FT]

        # --- self-attention ---
        xn = ln_t(h, n1g_col, 0, n1b_col, 0, BF16, f"xn1_{li}", "xn", bfacts)

        qkvw_l = g["qkvw"][li]
        wts = []
        for kt in range(FT):
            wt = wpool.tile([128, 3 * D], BF16, name=f"qkvw{li}_{kt}",
                            tag="wqkv", bufs=6)
            nc.sync.dma_start(wt[:], qkvw_l[kt * 128:(kt + 1) * 128, :])
            wts.append(wt)

        q_bf = bfacts.tile([128, FT * TOK], BF16, name=f"qr_{li}", tag="qbf")
        k_rope = bfacts.tile([128, FT * TOK], BF16, name=f"kr_{li}",
                             tag="krbf")
        for sec, dstw in ((0, q_bf), (1, k_rope)):
            for ft in range(FT):
                ps = psA.tile([128, TOK], F32, name=f"qk{li}", tag="mmps")
                col = sec * D + ft * 128
                for kt in range(FT):
                    nc.tensor.matmul(ps[:], wts[kt][:, col:col + 128],
                                     xn[:, kt * TOK:(kt + 1) * TOK],
                                     start=(kt == 0), stop=(kt == FT - 1))
                qf = tmp_pool.tile([128, TOK], F32, name=f"qkd{li}",
                                   tag="qkdrain", bufs=3)
                nc.scalar.activation(
                    qf[:], ps[:], AF.Identity,
                    bias=qkvb_col[:, 6 * sec + ft:6 * sec + ft + 1])
                rope_tile(qf[:], dstw[:, ft * TOK:(ft + 1) * TOK],
                          f"rope{li}_{sec}_{ft}")
        # v in token-major layout: v[tok, f] = sum_d xn_T[d, tok] Wv_T[d, f]
        vbias_sb = bias_pool.tile([128, D], F32, name=f"vb{li}", tag="vbias")
        nc.sync.dma_start(vbias_sb[:], g["vbias"][li])
        v_bf = bfacts.tile([128, 2 * D], BF16, name=f"v_{li}", tag="vloc")
        for half in range(2):
            for nch in range(2):
                ps = psA.tile([128, 384], F32, name=f"v{li}", tag="mmps")
                for kt in range(FT):
                    nc.tensor.matmul(
                        ps[:],
                        xn[:, kt * TOK + half * 128: kt * TOK + half * 128 + 128],
                        wts[kt][:, 2 * D + nch * 384: 2 * D + (nch + 1) * 384],
                        start=(kt == 0), stop=(kt == FT - 1))
                nc.vector.tensor_add(
                    v_bf[:, half * D + nch * 384: half * D + (nch + 1) * 384],
                    ps[:], vbias_sb[:, nch * 384:(nch + 1) * 384])

        # --- AllGather (k_rope, v) within the 4-core group ---
        snd = dram.tile([2 * KV_CHUNK], BF16, name=f"snd{li}", tag="snd")
        nc.sync.dma_start(
            snd[0:KV_CHUNK].rearrange("(kt p c) -> p kt c", kt=FT, p=128),
            k_rope[:].rearrange("p (kt c) -> p kt c", kt=FT))
        nc.sync.dma_start(
            snd[KV_CHUNK:2 * KV_CHUNK].rearrange("(hf p c) -> p hf c", hf=2,
                                                 p=128),
            v_bf[:].rearrange("p (hf c) -> p hf c", hf=2))
        rcv = dram.tile([8 * KV_CHUNK], BF16, name=f"rcv{li}", tag="rcv")
        if _no_cc:
            # timing-only mode: fake the gather with local DRAM copies
            for rk in range(4):
                nc.sync.dma_start(
                    rcv[rk * 2 * KV_CHUNK:(rk + 1) * 2 * KV_CHUNK], snd[:])
        else:
            nc.gpsimd.collective_compute(
                "AllGather", mybir.AluOpType.bypass, replica_groups=GROUPS,
                ins=[snd[:].opt()], outs=[rcv[:].opt()])

        v_all = kvpool.tile([128, NKT * D], BF16, name=f"vall{li}", tag="vall")
        # two 3D-AP DMAs (one per v half): src[rk, p, c] rank-major in rcv
        vsrc = rcv.rearrange("(rk hp p c) -> p rk hp c", rk=4, hp=4, p=128)
        vdst = v_all[:].rearrange("p (rk hf c) -> p rk hf c", rk=4, hf=2)
        for hf in range(2):
            nc.sync.dma_start(vdst[:, :, hf, :], vsrc[:, :, 2 + hf, :])

        def get_k(p, li=li, rcv=rcv):
            kp = kvpool.tile([128, S], BF16, name=f"kst{li}_{p}", tag="kst",
                             bufs=2)
            ksrc = rcv.rearrange("(rk half ft p c) -> p rk half ft c",
                                 rk=4, half=2, ft=FT, p=128)
            nc.sync.dma_start(
                kp[:].rearrange("p (rk c) -> p rk c", rk=4),
                ksrc[:, :, 0, p, :])
            return kp

        o_sa = attention(q_bf, get_k, v_all, NKT, S, f"sa{li}")
        h2 = acts.tile([128, FT * TOK], F32, name=f"h2_{li}", tag="h", bufs=3)
        for kt in range(FT):
            sl = slice(kt * TOK, (kt + 1) * TOK)
            nc.vector.tensor_add(h2[:, sl], h[:, sl], o_sa[:, sl])

        # --- cross-attention ---
        n2g_col = bcol[:, ON2G:ON2G + FT]
        n2b_col = bcol[:, ON2B:ON2B + FT]
        cab_col = bcol[:, OCAB:OCAB + 18]
        xn2 = ln_t(h2, n2g_col, 0, n2b_col, 0, BF16, f"xn2_{li}", "xn",
                   bfacts)

        caw_l = g["caw"][li]
        cwts = []
        for kt in range(FT):
            wt = wpool.tile([128, 3 * D], BF16, name=f"caw{li}_{kt}",
                            tag="wqkv", bufs=6)
            nc.sync.dma_start(wt[:], caw_l[kt * 128:(kt + 1) * 128, :])
            cwts.append(wt)

        caq_bf = bfacts.tile([128, FT * TOK], BF16, name=f"caq_{li}",
                             tag="qbf")
        for ft in range(FT):
            ps = psA.tile([128, TOK], F32, name=f"caq{li}", tag="mmps")
            for kt in range(FT):
                nc.tensor.matmul(ps[:], cwts[kt][:, ft * 128:(ft + 1) * 128],
                                 xn2[:, kt * TOK:(kt + 1) * TOK],
                                 start=(kt == 0), stop=(kt == FT - 1))
            nc.scalar.activation(caq_bf[:, ft * TOK:(ft + 1) * TOK], ps[:],
                                 AF.Identity, bias=cab_col[:, ft:ft + 1])
        cak_bf = bfacts.tile([128, FT * T_TXT], BF16, name=f"cak_{li}",
                             tag="krbf")
        for ft in range(FT):
            ps = psA.tile([128, T_TXT], F32, name=f"cak{li}", tag="mmps")
            col = D + ft * 128
            for kt in range(FT):
                nc.tensor.matmul(ps[0:128, :], cwts[kt][:, col:col + 128],
                                 text_bf[:, kt * T_TXT:(kt + 1) * T_TXT],
                                 start=(kt == 0), stop=(kt == FT - 1))
            nc.scalar.activation(cak_bf[:, ft * T_TXT:(ft + 1) * T_TXT],
                                 ps[0:128, :], AF.Identity,
                                 bias=cab_col[:, 6 + ft:7 + ft])
        cavb_sb = bias_pool.tile([128, D], F32, name=f"cavb{li}", tag="vbias")
        nc.sync.dma_start(cavb_sb[:], g["cavbias"][li])
        cav_bf = bfacts.tile([128, D], BF16, name=f"cav_{li}", tag="vloc")
        for nch in range(2):
            ps = psA.tile([128, 384], F32, name=f"cav{li}", tag="mmps")
            for kt in range(FT):
                nc.tensor.matmul(
                    ps[:], text_bf[:, kt * T_TXT:(kt + 1) * T_TXT],
                    cwts[kt][:, 2 * D + nch * 384: 2 * D + (nch + 1) * 384],
                    start=(kt == 0), stop=(kt == FT - 1))
            nc.vector.tensor_add(cav_bf[:, nch * 384:(nch + 1) * 384], ps[:],
                                 cavb_sb[:, nch * 384:(nch + 1) * 384])

        o_ca = attention(caq_bf,
                         lambda p: cak_bf[:, p * T_TXT:(p + 1) * T_TXT],
                         cav_bf, 1, T_TXT, f"ca{li}")

        caob_col = bcol[:, OCAOB:OCAOB + FT]
        oww = wpool.tile([128, FT * D], BF16, name=f"caow{li}",
                         tag="wsmall", bufs=1)
        nc.sync.dma_start(
            oww[:].rearrange("p (kt c) -> p kt c", kt=FT),
            g["caow"][li].rearrange("(kt p) c -> p kt c", p=128))
        h3 = acts.tile([128, FT * TOK], F32, name=f"h3_{li}", tag="h", bufs=3)
        for ft in range(FT):
            ps = psA.tile([128, TOK], F32, name=f"cao{li}", tag="mmps")
            for kt in range(FT):
                nc.tensor.matmul(ps[:],
                                 oww[:, kt * D + ft * 128:
                                     kt * D + (ft + 1) * 128],
                                 o_ca[:, kt * TOK:(kt + 1) * TOK],
                                 start=(kt == 0), stop=(kt == FT - 1))
            t = tmp_pool.tile([128, TOK], F32, name=f"cao_d{li}", tag="drain", bufs=4)
            nc.scalar.activation(t[:], ps[:], AF.Identity,
                                 bias=caob_col[:, ft:ft + 1])
            sl = slice(ft * TOK, (ft + 1) * TOK)
            nc.vector.tensor_add(h3[:, sl], h2[:, sl], t[:])

        # --- gated MLP (two DF halves, fc2 accumulated in SBUF) ---
        n3g_col = bcol[:, ON3G:ON3G + FT]
        n3b_col = bcol[:, ON3B:ON3B + FT]
        fc1b_col = bcol[:, OF1B:OF1B + 2 * FTH]
        gateb_col = bcol[:, OGB:OGB + 2 * FTH]
        fc2b_col = bcol[:, OF2B:OF2B + FT]
        xn3 = ln_t(h3, n3g_col, 0, n3b_col, 0, BF16, f"xn3_{li}", "xn",
                   bfacts)

        acc = acts.tile([128, FT * TOK], F32, name=f"acc_{li}", tag="acc",
                        bufs=1)
        for half in range(2):
            w1 = []
            wg = []
            for kt in range(FT):
                wt = wpool.tile([128, DFH], BF16, name=f"f1w{li}_{half}_{kt}",
                                tag="wmlp", bufs=7)
                nc.sync.dma_start(
                    wt[:], g["fc1w"][li][kt * 128:(kt + 1) * 128,
                                         half * DFH:(half + 1) * DFH])
                w1.append(wt)
            up_h = bfacts.tile([128, FTH * TOK], BF16, name=f"up{li}_{half}",
                               tag="up")
            for ft in range(FTH):
                ps = psA.tile([128, TOK], F32, name=f"f1{li}", tag="mmps")
                for kt in range(FT):
                    nc.tensor.matmul(ps[:],
                                     w1[kt][:, ft * 128:(ft + 1) * 128],
                                     xn3[:, kt * TOK:(kt + 1) * TOK],
                                     start=(kt == 0), stop=(kt == FT - 1))
                fi = half * FTH + ft
                nc.scalar.activation(up_h[:, ft * TOK:(ft + 1) * TOK], ps[:],
                                     AF.Gelu, bias=fc1b_col[:, fi:fi + 1])
            for kt in range(FT):
                wt = wpool.tile([128, DFH], BF16, name=f"gw{li}_{half}_{kt}",
                                tag="wmlp", bufs=7)
                nc.sync.dma_start(
                    wt[:], g["gatew"][li][kt * 128:(kt + 1) * 128,
                                          half * DFH:(half + 1) * DFH])
                wg.append(wt)
            gate_h = bfacts.tile([128, FTH * TOK], BF16,
                                 name=f"gate{li}_{half}", tag="gate", bufs=1)
            for ft in range(FTH):
                ps = psA.tile([128, TOK], F32, name=f"gt{li}", tag="mmps")
                for kt in range(FT):
                    nc.tensor.matmul(ps[:],
                                     wg[kt][:, ft * 128:(ft + 1) * 128],
                                     xn3[:, kt * TOK:(kt + 1) * TOK],
                                     start=(kt == 0), stop=(kt == FT - 1))
                fi = half * FTH + ft
                nc.scalar.activation(gate_h[:, ft * TOK:(ft + 1) * TOK],
                                     ps[:], AF.Sigmoid,
                                     bias=gateb_col[:, fi:fi + 1])
            for ft in range(FTH):
                sl = slice(ft * TOK, (ft + 1) * TOK)
                nc.vector.tensor_mul(up_h[:, sl], up_h[:, sl], gate_h[:, sl])
            w2w = wpool.tile([128, FTH * D], BF16, name=f"f2w{li}_{half}",
                             tag="wmlp2", bufs=1)
            nc.sync.dma_start(
                w2w[:].rearrange("p (kt c) -> p kt c", kt=FTH),
                g["fc2w"][li][half * DFH:(half + 1) * DFH, :].rearrange(
                    "(kt p) c -> p kt c", p=128))
            for ft in range(FT):
                ps = psA.tile([128, TOK], F32, name=f"f2{li}", tag="mmps")
                for kt in range(FTH):
                    nc.tensor.matmul(ps[:],
                                     w2w[:, kt * D + ft * 128:
                                         kt * D + (ft + 1) * 128],
                                     up_h[:, kt * TOK:(kt + 1) * TOK],
                                     start=(kt == 0), stop=(kt == FTH - 1))
                sl = slice(ft * TOK, (ft + 1) * TOK)
                if half == 0:
                    nc.scalar.activation(acc[:, sl], ps[:], AF.Identity,
                                         bias=fc2b_col[:, ft:ft + 1])
                else:
                    t = tmp_pool.tile([128, TOK], F32, name=f"f2d{li}",
                                      tag="drain", bufs=4)
                    nc.scalar.copy(t[:], ps[:])
                    nc.vector.tensor_add(acc[:, sl], acc[:, sl], t[:])

        h_new = acts.tile([128, FT * TOK], F32, name=f"h4_{li}", tag="h",
                          bufs=3)
        for ft in range(FT):
            sl = slice(ft * TOK, (ft + 1) * TOK)
            nc.vector.tensor_add(h_new[:, sl], h3[:, sl], acc[:, sl])
        h = h_new

    # ---------------- epilogue ----------------
    pob_col = ppk[:, FT:2 * FT]
    h_bf = bfacts.tile([128, FT * TOK], BF16, name="h_final", tag="xn")
    for kt in range(FT):
        sl = slice(kt * TOK, (kt + 1) * TOK)
        nc.scalar.copy(h_bf[:, sl], h[:, sl])
    poww = wpool.tile([128, FT * D], BF16, name="pow_sb", tag="wsmall",
                      bufs=1)
    nc.sync.dma_start(poww[:].rearrange("p (kt c) -> p kt c", kt=FT),
                      g["pow"].rearrange("(kt p) c -> p kt c", p=128))
    for ft in range(FT):
        ps = psA.tile([128, TOK], F32, name="po_ps", tag="mmps")
        for kt in range(FT):
            nc.tensor.matmul(ps[:],
                             poww[:, kt * D + ft * 128: kt * D + (ft + 1) * 128],
                             h_bf[:, kt * TOK:(kt + 1) * TOK],
                             start=(kt == 0), stop=(kt == FT - 1))
        t = tmp_pool.tile([128, TOK], F32, name="po_d", tag="drain", bufs=4)
        nc.scalar.activation(t[:], ps[:], AF.Identity,
                             bias=pob_col[:, ft:ft + 1])
        xs = tmp_pool.tile([128, TOK], F32, name="po_xs", tag="drain", bufs=4)
        nc.sync.dma_start(xs[:], xskip_dram[ft * 128:(ft + 1) * 128, :])
        o = tmp_pool.tile([128, TOK], F32, name="po_o", tag="drain", bufs=4)
        nc.vector.tensor_add(o[:], xs[:], t[:])
        nc.sync.dma_start(out_t[ft * 128:(ft + 1) * 128, :], o[:])


# ---------------------------------------------------------------------------
# host side
# ---------------------------------------------------------------------------

def _silu(x):
    return x / (1.0 + np.exp(-x))


def _host_prep(inputs, n_layers=L):
    f32 = lambda a: np.ascontiguousarray(np.asarray(a, np.float32))
    bf = lambda a: np.ascontiguousarray(np.asarray(a, np.float32)).astype(
        ml_dtypes.bfloat16)

    x = f32(inputs["x"])
    text_emb = f32(inputs["text_emb"])
    t = np.asarray(inputs["t"])

    temb = f32(inputs["t_emb_table"])[t]
    temb = _silu(temb @ f32(inputs["te_w1"]).T + f32(inputs["te_b1"]))
    temb = temb @ f32(inputs["te_w2"]).T + f32(inputs["te_b2"])
    tmean = text_emb.mean(axis=1)
    ada_t = _silu(temb) @ f32(inputs["ada_time_w"]).T + f32(inputs["ada_time_b"])
    ada_x = _silu(tmean) @ f32(inputs["ada_text_w"]).T + f32(inputs["ada_text_b"])
    tsc, tsh = ada_t[:, :D], ada_t[:, D:]
    xsc, xsh = ada_x[:, :D], ada_x[:, D:]
    scale = (1.0 + tsc + xsc).astype(np.float32)
    shift = (tsh + xsh).astype(np.float32)

    inv_freq = 1.0 / (10000.0 ** (np.arange(0, HD, 2, dtype=np.float32) / HD))
    pos = np.arange(S, dtype=np.float32)[:, None] * inv_freq[None, :]
    ang = np.concatenate([pos, pos], axis=1)  # [S, 64]
    cos_t = np.tile(np.cos(ang).T, (2, 1)).astype(np.float32)  # [128, S]
    sin_t = np.tile(np.sin(ang).T, (2, 1)).astype(np.float32)

    lw = _prep_weights(inputs, n_layers)

    in_maps = []
    for c in range(N_CORES):
        b, j = c // 4, c % 4
        rows = slice(j * TOK, (j + 1) * TOK)
        m = {}
        m["x_t"] = np.ascontiguousarray(x[b, rows].T)
        m["text_t"] = np.ascontiguousarray(text_emb[b].T).astype(
            ml_dtypes.bfloat16)
        m["ada_scale"] = scale[b]
        m["ada_shift"] = shift[b]
        m["cos_t"] = np.ascontiguousarray(cos_t[:, rows])
        m["sin_t"] = np.ascontiguousarray(sin_t[:, rows])
        in_maps.append(m)
    return in_maps, lw


def _prep_weights(inputs, n_layers=L):
    f32 = lambda a: np.ascontiguousarray(np.asarray(a, np.float32))
    bf = lambda a: np.ascontiguousarray(np.asarray(a, np.float32)).astype(
        ml_dtypes.bfloat16)
    lw = {}
    tr = lambda w: np.ascontiguousarray(np.swapaxes(f32(w), -1, -2))
    bcast_bias = lambda b: np.ascontiguousarray(np.broadcast_to(
        f32(b)[:, None, :], (n_layers, 128, D))).astype(np.float32)
    lw["qkvw"] = bf(tr(inputs["qkv_w"][:n_layers]))
    lw["vbias"] = bcast_bias(inputs["qkv_b"][:n_layers, 2 * D:])
    lw["caw"] = bf(tr(inputs["ca_qkv_w"][:n_layers]))
    lw["cavbias"] = bcast_bias(inputs["ca_qkv_b"][:n_layers, 2 * D:])
    lw["caow"] = bf(tr(inputs["ca_out_w"][:n_layers]))
    lw["fc1w"] = bf(tr(inputs["fc1_w"][:n_layers]))
    lw["gatew"] = bf(tr(inputs["gate_w"][:n_layers]))
    lw["fc2w"] = bf(tr(inputs["fc2_w"][:n_layers]))
    lw["piw"] = bf(f32(inputs["proj_in_w"]).T)
    lw["pow"] = bf(f32(inputs["proj_out_w"]).T)

    def cols(vec):
        return np.asarray(vec, np.float32).reshape(-1, 128).T  # [128, k]

    packs = []
    for li in range(n_layers):
        packs.append(np.concatenate([
            cols(inputs["qkv_b"][li]),          # 18
            cols(inputs["ca_qkv_b"][li]),       # 18
            cols(inputs["ca_out_b"][li]),       # 6
            cols(inputs["fc1_b"][li]),          # 24
            cols(inputs["gate_b"][li]),         # 24
            cols(inputs["fc2_b"][li]),          # 6
            cols(inputs["n1_g"][li]), cols(inputs["n1_b"][li]),
            cols(inputs["n2_g"][li]), cols(inputs["n2_b"][li]),
            cols(inputs["n3_g"][li]), cols(inputs["n3_b"][li]),
        ], axis=1))
    lw["bpack"] = np.ascontiguousarray(np.stack(packs)).astype(np.float32)
    lw["ppack"] = np.ascontiguousarray(np.concatenate([
        cols(inputs["proj_in_b"]), cols(inputs["proj_out_b"])],
        axis=1)).astype(np.float32)
    return lw


def kernel(**inputs):
    n_layers = _CACHE.get("n_layers", L)
    in_maps, lw = _host_prep(inputs, n_layers)
    if "nc" not in _CACHE:
        _CACHE["nc"] = build_program(n_layers, lw)
    nc = _CACHE["nc"]
    res = run_bass_kernel_spmd(nc, in_maps, core_ids=list(range(N_CORES)),
                               **_CACHE.get("run_kwargs", {}))
    _CACHE["last_result"] = res
    out = np.empty((B, S, D), np.float32)
    for c in range(N_CORES):
        b, j = c // 4, c % 4
        out[b, j * TOK:(j + 1) * TOK] = res.results[c]["out_t"].T
    return out
